# revision 4
# baseline (speedup 1.0000x reference)
"""Trainium2 Bass kernel for batched nearest-neighbor min-distance.

Problem: for each row u of U_z [16384, 256], compute
    min_{l in L_z [8192, 256]} ||u - l||_2
Strategy (8 NeuronCores, data-parallel over rows of U_z, L_z replicated;
`pred` is unused by the reference and ignored):
  d2(u,l) = ||u||^2 + ||l||^2 - 2 u.l
v3: fp8(e4m3) DoubleRowSwInterleave matmuls (as v2) + a CUSTOM DVE uop
(MIN3_PB_ANT, registered at runtime into concourse.dve_ops) that breaks the
two-engine consumer bound:
  Per core (2048 U cols), 64 L-tiles of [128 Lrows x 2048 Ucols] fp32 PSUM.
  v2 consumed each tile with either an ACT conv (2.0us) + DVE fp16 merge
  (1.13us) or a DVE fused stt (2.26us); LP-optimal mix ~92us/core of engine
  time (both engines saturated -- measured ~90-115us slope).
  v3 consumes tiles in PAIRS: even tile -> ACT conv (bias=l2c, fp32 PSUM ->
  bf16) written STRIDE-2 into the odd slots of an interleaved buffer
  Z = [(rmin_0, conv_0), (rmin_1, conv_1), ...]; odd tile -> ONE custom DVE
  op in 2X_1PORT mode: each cycle port0 reads the 32b pair (rmin_i, conv_i),
  port1 reads the 32b fp32 PSUM word whose HIGH half is bf16(psum_i)
  (SRC_1_HI), and the 8-stage datapath computes
      r' = min(rmin_i, conv_i, bf16(psum_i) + l2c)
  writing (r', r') back in place. One 2258ns DVE op thus retires TWO tiles
  (vs 2258ns for ONE in v2): DVE 32x2258 = 72us, ACT 32x2000 = 64us.
  Two Z buffers (parity) keep ACT/DVE overlapped; their running mins fold in
  the tail. perf_max=1 must be set on the instruction (stock _custom_dve
  hardcodes 0, which pins custom ops to the 1x slot); the 1x slot holds a
  MAX_NEG sentinel so a silent mode fallback fails loudly in rel-err.
  bf16 (not fp16) everywhere on the consumer side: the PSUM high-half trick
  IS bf16 truncation. Adds ~0.1-0.4% rel err on top of v2's fp8-input
  ~0.92%; gate is 2e-2.
All input DMAs on the sync-engine HWDGE queue only (splitting onto the ACT
queue serialized the pipeline -- v2 finding). Dummy matmuls burn the HAM
cold-clock window during the DMA head (v2 finding).
"""

import numpy as np

N, M, D = 16384, 8192, 256
CORES = 8
C_SHIFT = 256.0

_COMPILED = {}

# --- custom DVE op MIN3_PB_ANT (see module docstring) ---------------------- #

_MIN3_NAME = "MIN3_PB_ANT"


def _min3_reference(in0, in1, s0, s1, imm2):
    """CoreSim/interp semantics: in0 = interleaved (rmin, conv) pairs; in1 =
    bf16 bitcast of the fp32 PSUM tile (odd elements = bf16 truncation);
    out pair <- (r', r') with r' = min(rmin, conv, bf16(psum) + s0)."""
    x = np.asarray(in0, np.float32)
    p = np.asarray(in1, np.float32)
    P = x.shape[0]
    x2 = x.reshape(P, -1, 2)
    bias = np.asarray(s0, np.float32).reshape(-1, 1)
    r = np.minimum(np.minimum(x2[:, :, 0], x2[:, :, 1]),
                   p.reshape(P, -1, 2)[:, :, 1] + bias)
    out = np.empty_like(x2)
    out[:, :, 0] = r
    out[:, :, 1] = r
    return out.reshape(x.shape)


def _register_min3():
    """Register MIN3_PB_ANT in concourse.dve_ops; idempotent."""
    import concourse.dve_ops as dve_ops
    from concourse.dve_spec import C0, Spec, Src0, Src1, minn
    from concourse.dve_uop import (
        ENABLE,
        AluInp,
        AluOp,
        DveOpSpec,
        InpSel,
        OutPath,
        OutSel,
        Trigger,
        UopConfig,
    )

    for op in dve_ops.OPS:
        if op.name == _MIN3_NAME:
            return op

    def pair_uop():
        u = UopConfig()
        u.enable_input(InpSel.SRC_0, 1)       # chain0: rmin_i
        u.enable_input(InpSel.SRC_1_HI, 2)    # chain1: bf16(psum_i)
        u.enable_input(InpSel.CONST_0, 3)     # chain2: bias
        u.enable_input(InpSel.SRC_0_HI, 4)    # chain3: conv_i
        b = u.datapath_config
        b[0].enable_alu(AluOp.ADD, AluInp.PREV_DELAY_1, AluInp.PREV_DELAY_2)
        b[0].pass_through_delay(0, 3)
        b[1].enable_alu(AluOp.MIN, AluInp.PREV_DELAY_0, AluInp.PREV_ALU_OUT)
        b[1].pass_through_delay(3)
        b[2].enable_alu(AluOp.MIN, AluInp.PREV_DELAY_3, AluInp.PREV_ALU_OUT)
        for k in range(3, 8):
            b[k].pass_through_alu()
        u.require_inp0 = ENABLE
        u.require_inp1 = ENABLE
        u.trigger = (Trigger.SRC_TENSOR_DONE, Trigger.NONE, Trigger.NONE)
        u.enable_output(OutSel.ALU_OUT, OutPath.WR0_LO)
        u.enable_output(OutSel.ALU_OUT, OutPath.WR0_HI)
        return u

    def sentinel_uop():
        # 1x slot: write MAX_NEG so a silent fallback out of 2X mode is
        # unmistakable (output collapses to 0 distances -> rel err ~1).
        u = UopConfig()
        u.enable_input(InpSel.MAX_NEG, 1)
        b = u.datapath_config
        b[0].enable_alu(AluOp.BYPASS, AluInp.PREV_DELAY_0, AluInp.PREV_DELAY_0)
        for k in range(1, 8):
            b[k].pass_through_alu()
        u.require_inp0 = ENABLE
        u.require_inp1 = ENABLE
        u.trigger = (Trigger.SRC_TENSOR_DONE, Trigger.NONE, Trigger.NONE)
        u.enable_output(OutSel.ALU_OUT, OutPath.WR0_LO)
        return u

    row = dve_ops._CUSTOM_DVE_ROW_BASE + len(dve_ops.OPS)
    assert row < 0x20

    spec_obj = DveOpSpec(
        name=_MIN3_NAME,
        opcode=row,
        uops=[sentinel_uop()],
        uops_2x=[pair_uop()],
        perf_max=1,
        rd1_en=True,
    )

    class _HandOp:
        name = _MIN3_NAME
        spec = Spec(body=minn(minn(Src0, Src1), C0), reference=_min3_reference)
        subdim = False
        perf_en: dict = {}

        def compile(self, ver):
            assert ver == "v3", f"{_MIN3_NAME} only wired for v3, got {ver}"
            return spec_obj

    op = _HandOp()
    dve_ops.OPS.append(op)
    dve_ops.CUSTOM_DVE_SPECS[_MIN3_NAME] = op.spec
    dve_ops._SUB_OPCODE_FOR_NAME[_MIN3_NAME] = row
    return op


def _emit_min3(nc, out, in0, in1, s0):
    """Emit MIN3_PB_ANT with perf_max=1 (2X slot reachable; stock
    _custom_dve hardcodes perf_max=0 which pins custom ops to 1x)."""
    import concourse.bass_isa as bass_isa
    import concourse.dve_ops as dve_ops
    from concourse import mybir

    op = _register_min3()
    v = nc.vector
    bass = v.bass
    if op.name not in bass.m.ant_custom_dve_ops:
        bass.m.ant_custom_dve_ops = sorted(
            {*bass.m.ant_custom_dve_ops, op.name})
    shape = bass_isa.CustomDveShape.TTSS
    isa_opcode = bass.isa.Opcode[
        f"NEURON_ISA_TPB_OPCODE_CUSTOM_DVE_ANT_{shape.slot()}"
    ].value
    ins = [
        v.lower_ap(in0, for_isa=True, opt=True),
        v.lower_ap(in1, for_isa=True, opt=True),
        v.lower_ap(s0, for_isa=True),
        mybir.ImmediateValue(dtype=mybir.dt.float32, value=0.0),
    ]
    outs = [v.lower_ap(out, for_isa=True, opt=True)]
    return v.add_instruction(
        bass_isa.InstCustomDveAnt(
            name=bass.get_next_instruction_name(),
            op_name=op.name,
            rd1_en=True,
            subdim=0,
            imm2=0.0,
            shape=shape,
            row=dve_ops.get_dve_sub_opcode(op.name),
            isa_opcode=isa_opcode,
            perf_max=1,
            ins=ins,
            outs=outs,
        )
    )


def _build(ucols: int, m: int, pattern=None, debug: bool = False, rounds: int = 1,
           mm_mode: str = "drswi", **_ignored):
    """Build + compile the per-core Bass kernel.

    ucols:  number of U columns (rows of U_z) this core handles.
    m:      number of L rows (library size).
    rounds: repeat the whole computation this many times inside a hardware
            loop (benchmarking only -- slope between round counts isolates
            steady-state HW time from the host dispatch overhead).
    """
    from contextlib import ExitStack, nullcontext

    import concourse.bacc as bacc
    import concourse.tile as tile
    from concourse import mybir

    F32 = mybir.dt.float32
    BF16 = mybir.dt.bfloat16
    FP8 = mybir.dt.float8e4
    AF = mybir.ActivationFunctionType
    ALU = mybir.AluOpType
    DR = (mybir.MatmulPerfMode.DoubleRowSwInterleave if mm_mode == "drswi"
          else mybir.MatmulPerfMode.DoubleRow)

    ltiles = m // 128
    assert ucols % 512 == 0 and m % 128 == 0
    assert ltiles % 4 == 0

    nc = bacc.Bacc("TRN2", target_bir_lowering=False, debug=debug)

    blocks = ucols // 32
    ut_d = nc.dram_tensor("ut", [128, 2, ucols], FP8, kind="ExternalInput").ap()
    lt_shape = [128, 2 * m] if mm_mode == "drswi" else [128, 2, m]
    lt_d = nc.dram_tensor("lt", lt_shape, FP8, kind="ExternalInput").ap()
    l2c_d = nc.dram_tensor("l2c", [128, ltiles], F32, kind="ExternalInput").ap()
    u2c_d = nc.dram_tensor("u2c", [32, blocks], F32, kind="ExternalInput").ap()
    out_d = nc.dram_tensor("out", [32, blocks], F32, kind="ExternalOutput").ap()

    with tile.TileContext(nc) as tc, ExitStack() as ctx:
        const_pool = ctx.enter_context(tc.tile_pool(name="const", bufs=1))
        psum_pool = ctx.enter_context(
            tc.tile_pool(name="psum", bufs=2, space="PSUM"))

        ut_sb = const_pool.tile([128, 2, ucols], FP8, name="utsb")
        lt_sb = const_pool.tile(lt_shape, FP8, name="ltsb")
        l2c = const_pool.tile([128, ltiles], F32, name="l2c")
        u2c = const_pool.tile([32, blocks], F32, name="u2c")
        # Interleaved (running-min, conv-staging) pair buffers; two for
        # ACT/DVE overlap (per-Z serial chain conv -> min3 -> conv ...).
        z0 = const_pool.tile([128, 2 * ucols], BF16, name="z0")
        z1 = const_pool.tile([128, 2 * ucols], BF16, name="z1")
        zs = (z0, z1)
        zviews = tuple(z.rearrange("p (n two) -> p n two", two=2) for z in zs)

        loop_cm = tc.For_i(0, rounds, 1) if rounds > 1 else nullcontext()
        ctx.enter_context(loop_cm)

        # Small + U loads first so the main loop can start on L-chunk 0.
        nc.sync.dma_start(l2c[:], l2c_d[:])
        nc.sync.dma_start(u2c[:], u2c_d[:])
        nc.sync.dma_start(ut_sb[:], ut_d[:])
        if mm_mode == "drswi":
            CH = min(2048, 2 * m)
            for c0 in range(0, 2 * m, CH):
                nc.sync.dma_start(lt_sb[:, c0:c0 + CH], lt_d[:, c0:c0 + CH])
        else:
            CH = min(1024, m)
            for c0 in range(0, m, CH):
                nc.sync.dma_start(lt_sb[:, :, c0:c0 + CH], lt_d[:, :, c0:c0 + CH])

        # Dummy matmuls during the DMA head: burn the HAM cold-clock window
        # (PE at 1.2 GHz until ~3.4us of sustained activity) on scratch
        # weights so the real tiles start at 2.4 GHz.
        wght = const_pool.tile([128, 256], FP8, name="wght")
        wsrc = const_pool.tile([128, 2, 512], FP8, name="wsrc")
        nc.vector.memset(wght.bitcast(F32)[:], 1.0)
        nc.vector.memset(wsrc.bitcast(F32)[:], 1.0)
        wpsum = psum_pool.tile([128, ucols], F32, name="psum", tag="psum")
        for _ in range(8):
            nc.tensor.matmul(wpsum[:, 0:512], wght[:], wsrc[:],
                             start=True, stop=True, perf_mode=DR)

        nc.vector.memset(z0[:], 30000.0)
        nc.vector.memset(z1[:], 30000.0)

        for lt in range(ltiles):
            bias = l2c[:, lt:lt + 1]
            psum = psum_pool.tile([128, ucols], F32, name="psum", tag="psum")
            if mm_mode == "drswi":
                lhsT = lt_sb[:, lt * 256:(lt + 1) * 256]
            else:
                lhsT = lt_sb[:, :, lt * 128:(lt + 1) * 128]
            for s0 in range(0, ucols, 512):
                nc.tensor.matmul(
                    psum[:, s0:s0 + 512],
                    lhsT,
                    ut_sb[:, :, s0:s0 + 512],
                    start=True,
                    stop=True,
                    perf_mode=DR,
                )
            zi = (lt // 2) % 2
            if lt % 2 == 0:
                # ACT: conv = bf16(psum + l2c) into the odd (staging) slots.
                nc.scalar.activation(zviews[zi][:, :, 1], psum[:],
                                     AF.Identity, bias=bias, scale=1.0)
            else:
                # Custom DVE op: one 2X pass retires this PSUM tile AND the
                # staged conv: rmin = min(rmin, conv, bf16(psum) + l2c).
                _emit_min3(nc, zs[zi][:], zs[zi][:],
                           psum.bitcast(BF16)[:], bias)

        # Fold the two Z chains' running mins (even slots) -> contiguous.
        rmin = const_pool.tile([128, ucols], BF16, name="rmin")
        nc.vector.tensor_tensor(rmin[:], zviews[0][:, :, 0],
                                zviews[1][:, :, 0], op=ALU.min)

        # Partition reduction: transpose every 32x32 block, min over the
        # free dim within each block -> red[32g + i, b] = min over
        # partitions {32g..32g+31} of column 32b + i. Then two tree levels
        # across the four partition groups (base partitions must be
        # 32-aligned and equal for DVE TT, so realign with tiny DMAs).
        tr = const_pool.tile([128, ucols], BF16, name="tr")
        nc.vector.transpose(tr[:], rmin[:])
        red = const_pool.tile([128, blocks], BF16, name="red")
        nc.vector.tensor_reduce(
            red[:], tr.rearrange("p (b j) -> p b j", j=32),
            axis=mybir.AxisListType.X, op=ALU.min,
        )
        half = const_pool.tile([64, blocks], BF16, name="half")
        nc.sync.dma_start(half[:], red[64:128, :])
        nc.vector.tensor_tensor(red[:64, :], red[:64, :], half[:, :], op=ALU.min)
        quart = const_pool.tile([32, blocks], BF16, name="quart")
        nc.sync.dma_start(quart[:], red[32:64, :])
        nc.vector.tensor_tensor(red[:32, :], red[:32, :], quart[:, :], op=ALU.min)
        pmin = red[:32, :]
        d2 = const_pool.tile([32, blocks], F32, name="d2")
        nc.vector.tensor_tensor(d2[:], pmin[:], u2c[:], op=ALU.add)
        nc.vector.tensor_scalar_max(d2[:], d2[:], 0.0)
        outt = const_pool.tile([32, blocks], F32, name="outt")
        nc.scalar.activation(outt[:], d2[:], AF.Sqrt)
        nc.sync.dma_start(out_d[:], outt[:])

    nc.compile()
    return nc


def _get_compiled(ucols: int, m: int):
    key = (ucols, m)
    if key not in _COMPILED:
        _COMPILED[key] = _build(ucols, m)
    return _COMPILED[key]


def _prep_inputs(U: np.ndarray, L: np.ndarray, mm_mode: str = "drswi"):
    """Host-side sharding / layout prep (transpose, -2 scale, norm rows).

    Moving operand (U) DoubleRow layout: tile[p, i, x] = T[i*128 + p, x]
    for the transposed operand T [256, X] (logical K index = i*128 + p).
    Stationary operand (L) for DoubleRowSwInterleave: per L-tile, 256
    bytes per partition with w[p, 2*j + i] = LT[i*128 + p, tile*128 +
    (127 - j)] (pairs interleaved per column, columns reversed), so the
    hardware LDWEIGHTS is a contiguous read.
    """
    import ml_dtypes

    n, d = U.shape
    m = L.shape[0]
    ucols = n // CORES
    FP8 = ml_dtypes.float8_e4m3
    UTm2 = np.ascontiguousarray((-2.0 * U).T).reshape(2, 128, n)
    UTm2 = UTm2.transpose(1, 0, 2)  # [128, 2, n]
    LT3 = np.ascontiguousarray(L.T).reshape(2, 128, m)  # [i, p, dcol]
    if mm_mode == "drswi":
        # [i, p, tile, j'] with column reversal inside each 128-wide tile
        B = LT3.reshape(2, 128, m // 128, 128)[:, :, :, ::-1]
        # -> [p, tile, j', i] -> flatten to [128, 2*m]
        LT8 = np.ascontiguousarray(
            B.transpose(1, 2, 3, 0).reshape(128, 2 * m)).astype(FP8)
    else:
        LT8 = np.ascontiguousarray(LT3.transpose(1, 0, 2)).astype(FP8)
    l2 = (L.astype(np.float64) ** 2).sum(1).astype(np.float32)
    u2 = (U.astype(np.float64) ** 2).sum(1).astype(np.float32)
    l2cT = np.ascontiguousarray((l2 - C_SHIFT).reshape(m // 128, 128).T)
    u2c = u2 + C_SHIFT
    in_maps = []
    for i in range(CORES):
        sl = slice(i * ucols, (i + 1) * ucols)
        # Device output layout is [32, ucols//32] with column c = 32*b + i at
        # [i, b]; u2c must match that layout.
        u2c_dev = np.ascontiguousarray(u2c[sl].reshape(ucols // 32, 32).T)
        in_maps.append({
            "ut": np.ascontiguousarray(UTm2[:, :, sl]).astype(FP8),
            "lt": LT8,
            "l2c": l2cT,
            "u2c": u2c_dev,
        })
    return in_maps


def kernel(**inputs) -> np.ndarray:
    from concourse import bass_utils

    U = np.asarray(inputs["U_z"], dtype=np.float32)
    L = np.asarray(inputs["L_z"], dtype=np.float32)
    n = U.shape[0]
    m = L.shape[0]
    ucols = n // CORES
    nc = _get_compiled(ucols, m)
    in_maps = _prep_inputs(U, L)
    res = bass_utils.run_bass_kernel_spmd(nc, in_maps, list(range(CORES)))
    # Per-core output [32, ucols//32] holds column c = 32*b + i at [i, b].
    return np.concatenate(
        [np.ascontiguousarray(r["out"].T).reshape(-1) for r in res.results]
    ).astype(np.float32)


if __name__ == "__main__":
    # Smoke test with random data against a numpy reference.
    rng = np.random.default_rng(0)
    U = rng.standard_normal((N, D), dtype=np.float32)
    L = rng.standard_normal((M, D), dtype=np.float32)
    out = kernel(pred=None, U_z=U, L_z=L)
    d2 = (U * U).sum(1)[:, None] + (L * L).sum(1)[None, :] - 2.0 * U @ L.T
    exp = np.sqrt(np.maximum(d2, 0.0).min(1))
    rel = np.abs(out - exp) / np.maximum(np.abs(exp), 1e-9)
    print("max rel err:", rel.max())
